# revision 1
# baseline (speedup 1.0000x reference)
"""Trainium2 Bass kernel for the ChaosModulator recurrence.

Math (per (b,c) sequence, t = 0..4095):
    sigma_t = 3.5*z*(1-z) + 0.5*x_t
    z'      = 0.5*z + 0.25*(1 + tanh(sigma_t))        (clip is a no-op: z' in (0,1))
    u_t     = 0.5*x_t + 0.5*(2*z' - 1)

Reformulated with w = 2z-1 and state s_t = w_t + h_t (so w_{t+1} = 0.5*s_t):
    e_t = x_t - (7/16)*s_{t-1}^2
    h_t = tanh(0.5*e_t + 0.875)
    s_t = 0.5*s_{t-1} + h_t
    u_t = 0.25*s_t + 0.5*x_t

The map contracts with factor ~0.5/step, so each 128-step time block can be
computed independently after a 32-step warmup from an arbitrary state
(validated: fp64-exact at W=32).  This turns the serial t-loop into
32 independent chains per sequence -> wide [128 x 128] per-step ops.

Per step: 1 ACT op (tanh) + 2 DVE ops (stt s-update, fused CHAOS_E e-update).
States for output steps are written into dead X-tile slots; u is produced by
one bulk fused CHAOS_U op per batch.

Sharding: batch dim b (32) split 4-per-core across 8 cores; per core
2048 sequences x 4096 steps.
"""

import numpy as np

import concourse.bacc as bacc
import concourse.dve_ops as dve_ops
import concourse.mybir as mybir
from concourse.bass_utils import run_bass_kernel_spmd
from concourse.dve_spec import C0, C1, Spec, Src0, Src1, _has_src1, lower, sq
from concourse.dve_uop import DveOpSpec
from concourse.tile import TileContext

F32 = mybir.dt.float32
P = 128             # SBUF partitions
G = 16              # sequence groups per core (2048 = G*P)
T = 4096
B = 128             # output steps per block
W = 32              # warmup steps per block
L = B + W           # chain length
NBLK = T // B       # 64 blocks
BLK_PER_BATCH = 8
NBATCH = NBLK // BLK_PER_BATCH   # 8
NCOLS = BLK_PER_BATCH * G        # 128 chain-columns per batch
NSEQ = P * G        # 2048 sequences per core
NCORES = 8
XBUFS = 2           # batches in flight (SBUF: XBUFS * 80KB/partition)

_MULT = mybir.AluOpType.mult
_ADD = mybir.AluOpType.add


def _register_custom_ops():
    """Register the two fused DVE ops (idempotent)."""
    if "CHAOS_E" in dve_ops._SUB_OPCODE_FOR_NAME:
        by = {op.name: op for op in dve_ops.OPS}
        return by["CHAOS_E"], by["CHAOS_U"]

    spec_e = Spec(
        body=Src1 - C0 * sq(Src0),
        reference=lambda in0, in1, s0: in1 - s0 * in0 * in0,
    )
    spec_u = Spec(
        body=C0 * Src0 + C1 * Src1,
        reference=lambda in0, in1, s0, s1: s0 * in0 + s1 * in1,
    )
    ops = []
    for name, spec in (("CHAOS_E", spec_e), ("CHAOS_U", spec_u)):
        op = dve_ops.DveOp(name, spec, subdim=False, uops_sha={})
        dve_ops.OPS.append(op)
        dve_ops.CUSTOM_DVE_SPECS[name] = spec
        dve_ops._SUB_OPCODE_FOR_NAME[name] = (
            dve_ops._CUSTOM_DVE_ROW_BASE + len(dve_ops.OPS) - 1
        )
        # pin the uops sha self-consistently
        for ver in ("v3", "v4"):
            try:
                s = DveOpSpec(
                    name=name,
                    opcode=dve_ops.get_dve_sub_opcode(name),
                    uops=lower(spec, ver=ver),
                    rd1_en=_has_src1(spec),
                )
                op.uops_sha[ver] = s.sha(ver)
            except Exception:
                pass
        ops.append(op)
    return ops


def _build_nc():
    CHAOS_E, CHAOS_U = _register_custom_ops()
    nc = bacc.Bacc("TRN2", target_bir_lowering=False, debug=False)

    x = nc.dram_tensor("x", [NSEQ, T], F32, kind="ExternalInput")
    z0 = nc.dram_tensor("z0", [NSEQ], F32, kind="ExternalInput")
    u = nc.dram_tensor("u", [NSEQ, T], F32, kind="ExternalOutput")

    xr = x[:, :].rearrange("(g p) t -> p g t", p=P)    # [128, 16, 4096]
    ur = u[:, :].rearrange("(g p) t -> p g t", p=P)
    z0r = z0[:].rearrange("(g p) -> p g", p=P)         # [128, 16]

    with TileContext(nc) as tc:
        with (
            tc.tile_pool(name="xp", bufs=XBUFS) as xp,
            tc.tile_pool(name="sp", bufs=XBUFS) as sp,
            tc.tile_pool(name="cp", bufs=1) as cp,
        ):
            z0_t = cp.tile([P, G], F32)
            nc.sync.dma_start(out=z0_t[:, :], in_=z0r)
            # s_init = 4*z0 - 2  (so that w_0 = 0.5*s_init = 2*z0 - 1)
            s_init = cp.tile([P, G], F32)
            nc.vector.tensor_scalar(
                out=s_init[:, :], in0=z0_t[:, :],
                scalar1=4.0, scalar2=-2.0, op0=_MULT, op1=_ADD,
            )
            # per-partition bias for tanh(0.5*e + 0.875)
            bias_t = cp.tile([P, 1], F32)
            nc.vector.memset(bias_t[:, :], 0.875)

            for bt in range(NBATCH):
                # X tile: [c][k] layout, c = nl*G + g, k = chain step.
                # Slots hold raw x; slots 0..B-1 are progressively reused to
                # hold the state s_k (slot k-W) and finally u.
                Xt = xp.tile([P, NCOLS * L], F32, name=f"X{bt}", tag="X")
                Xv = Xt.rearrange("p (c k) -> p c k", k=L)

                for nl in range(BLK_PER_BATCH):
                    n = bt * BLK_PER_BATCH + nl
                    cs = nl * G
                    if n == 0:
                        # block 0 starts at t=-W: pad warmup with zeros
                        nc.vector.memset(Xv[:, cs:cs + G, 0:W], 0.0)
                        nc.sync.dma_start(
                            out=Xv[:, cs:cs + G, W:L], in_=xr[:, :, 0:B]
                        )
                    else:
                        t0 = n * B - W
                        nc.sync.dma_start(
                            out=Xv[:, cs:cs + G, :], in_=xr[:, :, t0:t0 + L]
                        )

                h_t = sp.tile([P, NCOLS], F32, name=f"h{bt}", tag="h")
                e_t = [
                    sp.tile([P, NCOLS], F32, name=f"e{bt}_{i}", tag=f"e{i}")
                    for i in range(2)
                ]
                s_t = [
                    sp.tile([P, NCOLS], F32, name=f"s{bt}_{i}", tag=f"s{i}")
                    for i in range(2)
                ]

                nc.vector.memset(s_t[0][:, :], 0.0)
                # e_0 = x_0 - (7/16)*0^2 = x_0
                nc.vector.tensor_copy(out=e_t[0][:, :], in_=Xv[:, :, 0])

                for k in range(L):
                    cur, nxt = k % 2, (k + 1) % 2
                    # h = tanh(0.5*e + 0.875)
                    nc.scalar.activation(
                        out=h_t[:, :], in_=e_t[cur][:, :],
                        func=mybir.ActivationFunctionType.Tanh,
                        bias=bias_t[:, :], scale=0.5,
                    )
                    # s' = 0.5*s + h ; for k>=W write into dead X slot k-W
                    s_prev = s_t[cur][:, :] if k <= W else Xv[:, :, k - 1 - W]
                    s_out = s_t[nxt][:, :] if k < W else Xv[:, :, k - W]
                    nc.vector.scalar_tensor_tensor(
                        out=s_out, in0=s_prev, scalar=0.5,
                        in1=h_t[:, :], op0=_MULT, op1=_ADD,
                    )
                    if bt == 0 and k == W - 1:
                        # block 0: replace warmup state with the true z0 state
                        nc.vector.tensor_copy(
                            out=s_t[nxt][:, 0:G], in_=s_init[:, :]
                        )
                        s_out = s_t[nxt][:, :]
                    if k < L - 1:
                        # e' = x_{k+1} - (7/16)*s'^2
                        nc.vector._custom_dve(
                            CHAOS_E, out=e_t[nxt][:, :], in0=s_out,
                            in1=Xv[:, :, k + 1], s0=0.4375,
                        )
                    # chunked bulk u = 0.25*s_j + 0.5*x_j for j in [k-16, k):
                    # s_j sits in slot j-W (last read by step j+1 <= k, done),
                    # x_j in slot j (destroyed at step j+W >= k+16, alive).
                    if k >= W + 16 and (k - W) % 16 == 0:
                        lo = k - W - 16
                        nc.vector._custom_dve(
                            CHAOS_U,
                            out=Xv[:, :, lo:lo + 16], in0=Xv[:, :, lo:lo + 16],
                            in1=Xv[:, :, k - 16:k], s0=0.25, s1=0.5,
                        )

                # final u chunk: j in [L-16, L)
                nc.vector._custom_dve(
                    CHAOS_U,
                    out=Xv[:, :, B - 16:B], in0=Xv[:, :, B - 16:B],
                    in1=Xv[:, :, L - 16:L], s0=0.25, s1=0.5,
                )

                for nl in range(BLK_PER_BATCH):
                    n = bt * BLK_PER_BATCH + nl
                    cs = nl * G
                    nc.sync.dma_start(
                        out=ur[:, :, n * B:(n + 1) * B],
                        in_=Xv[:, cs:cs + G, 0:B],
                    )

    nc.compile()
    return nc


_NC = None


def _get_nc():
    global _NC
    if _NC is None:
        _NC = _build_nc()
    return _NC


def kernel(x: np.ndarray, z0: np.ndarray) -> np.ndarray:
    x = np.ascontiguousarray(x, dtype=np.float32)      # (32, 512, 4096)
    z0 = np.ascontiguousarray(z0, dtype=np.float32)    # (32, 512)
    nc = _get_nc()

    in_maps = []
    for i in range(NCORES):
        xs = np.ascontiguousarray(x[4 * i:4 * (i + 1)].reshape(NSEQ, T))
        zs = np.ascontiguousarray(z0[4 * i:4 * (i + 1)].reshape(NSEQ))
        in_maps.append({"x": xs, "z0": zs})

    res = run_bass_kernel_spmd(nc, in_maps, core_ids=list(range(NCORES)))
    out = np.empty((32, 512, T), np.float32)
    for i in range(NCORES):
        out[4 * i:4 * (i + 1)] = res.results[i]["u"].reshape(4, 512, T)
    return out

